# revision 4
# baseline (speedup 1.0000x reference)
"""Trainium2 Bass kernel for nn_ConnectFourFeatures.

Reference computes out = concat([x] + [conv(x, f) for f in 8 filters], axis=1)
on x [131072, 3, 6, 7] fp32; each filter is channel-diagonal with 0/1 taps on a
line (horiz/vert/diag/anti, run lengths 3 and 5). Output [131072, 27, 6, 7].

Strategy (v2):
 - Pure data-parallel over 8 NeuronCores (batch sharded), no collectives.
 - fp16 end-to-end on device (correctness gate is 2e-2 rel; fp16 gives ~4e-4).
   Host converts x -> fp16 and upcasts the result; halves all HBM traffic.
 - Identity channels (0:3) are never touched on device: the host writes the
   exact fp32 x into the output. Device computes only the 24 conv channels.
 - Sample-PAIR interleaved innermost: SBUF layout [P, C, s2, H, W*2] where the
   last dim holds (w, pair) merged. Every shifted add is then a 3D free AP
   [C*s2, hs, 2*ws] (DVE ISA limit) whose innermost dim is step-1, >=2 elems,
   at even element offsets -> DVE 2x packed mode for 2-byte dtypes.
 - Per direction d: out[R+] = x + x[+d]; complement rows/cols/corners get
   their own small ops; out[R+ & R-] += x[-d]. l=5 filters: ACT base copy of
   the l=3 block + two in-place adds (+-2d). Vertical l5 adds go to GpSimd to
   offload DVE.
"""

import numpy as np

import concourse.bass as bass
import concourse.mybir as mybir
from concourse import bacc
from concourse.bass_utils import run_bass_kernel_spmd
from concourse.tile import TileContext

N_CORES = 8
H, W = 6, 7
CIN = 3
HW = H * W  # 42
P = 128
DIRS = [(0, 1), (1, 0), (1, 1), (1, -1)]  # horiz, vert, diag, anti
F16 = mybir.dt.float16


def _region(dh, dw):
    """(h, w) slices of out cells whose tap x[h+dh, w+dw] is in bounds."""
    return (slice(max(0, -dh), H - max(0, dh)),
            slice(max(0, -dw), W - max(0, dw)))


def _sh(s, d):
    return slice(s.start + d, s.stop + d)


def _w2(ws, dw=0):
    """w-range slice in the merged (w, pair) coordinate, shifted by dw."""
    return slice(2 * (ws.start + dw), 2 * (ws.stop + dw))


def _isect(a, b):
    return slice(max(a.start, b.start), min(a.stop, b.stop))


class NpEmitter:
    """Numpy emulation of the per-tile program (fp16 semantics like DVE)."""

    def add(self, out, a, b, eng="v"):
        out[...] = a + b

    def copy(self, out, a, eng="v"):
        out[...] = a


class BassEmitter:
    def __init__(self, nc):
        self.nc = nc

    def add(self, out, a, b, eng="v"):
        e = self.nc.gpsimd if eng == "g" else self.nc.vector
        e.tensor_add(out=out, in0=a, in1=b)

    def copy(self, out, a, eng="v"):
        if eng == "s":
            self.nc.scalar.copy(out=out, in_=a)
        else:
            self.nc.vector.tensor_copy(out=out, in_=a)


def emit_tile(em, xt, ot, pool_l5=(1,)):
    """xt [P, 3, s2, H, 14] -> ot [P, 24, s2, H, 14] (l3 chans 0:12, l5 chans
    12:24, direction order horiz/vert/diag/anti; (w, pair) merged last)."""
    full = slice(0, W)
    # l=3 filters
    for i, (dh, dw) in enumerate(DIRS):
        b3 = ot[:, 3 * i : 3 * i + 3]
        hp, wp = _region(dh, dw)
        hm, wm = _region(-dh, -dw)
        # op1: fresh write covering R+ : x + x[+d]
        em.add(b3[:, :, :, hp, _w2(wp)], xt[:, :, :, hp, _w2(wp)],
               xt[:, :, :, _sh(hp, dh), _w2(wp, dw)])
        # op2: complement of R+ gets x + x[-d] (corners: copy of x)
        if dh == 0:
            wc = slice(W - 1, W) if dw > 0 else slice(0, 1)
            em.add(b3[:, :, :, :, _w2(wc)], xt[:, :, :, :, _w2(wc)],
                   xt[:, :, :, :, _w2(wc, -dw)])
        elif dw == 0:
            hc = slice(H - 1, H) if dh > 0 else slice(0, 1)
            em.add(b3[:, :, :, hc, :], xt[:, :, :, hc, :],
                   xt[:, :, :, _sh(hc, -dh), :])
        else:
            hc = slice(H - 1, H)  # dh == 1 for both diagonal patterns
            wc = slice(W - 1, W) if dw > 0 else slice(0, 1)
            wr = slice(max(0, dw), W + min(0, dw))
            em.add(b3[:, :, :, hc, _w2(wr)], xt[:, :, :, hc, _w2(wr)],
                   xt[:, :, :, _sh(hc, -dh), _w2(wr, -dw)])
            hr = slice(1, H - 1)
            em.add(b3[:, :, :, hr, _w2(wc)], xt[:, :, :, hr, _w2(wc)],
                   xt[:, :, :, _sh(hr, -dh), _w2(wc, -dw)])
            oc = slice(W - 1 - wc.start, W - wc.start)  # untapped corners
            em.copy(b3[:, :, :, hc, _w2(oc)], xt[:, :, :, hc, _w2(oc)])
            em.copy(b3[:, :, :, 0:1, _w2(wc)], xt[:, :, :, 0:1, _w2(wc)])
        # op3: += x[-d] where both taps exist
        hi, wi = _isect(hp, hm), _isect(wp, wm)
        em.add(b3[:, :, :, hi, _w2(wi)], b3[:, :, :, hi, _w2(wi)],
               xt[:, :, :, _sh(hi, -dh), _w2(wi, -dw)])

    # l=5: base copy of the l3 result (ACT), then += x[+-2d]
    for i in range(4):
        em.copy(ot[:, 12 + 3 * i : 15 + 3 * i], ot[:, 3 * i : 3 * i + 3],
                eng="s")
    for i, (dh0, dw0) in enumerate(DIRS):
        b5 = ot[:, 12 + 3 * i : 15 + 3 * i]
        eng = "g" if i in pool_l5 else "v"
        for sgn in (2, -2):
            dh, dw = sgn * dh0, sgn * dw0
            hs, ws = _region(dh, dw)
            em.add(b5[:, :, :, hs, _w2(ws)], b5[:, :, :, hs, _w2(ws)],
                   xt[:, :, :, _sh(hs, dh), _w2(ws, dw)], eng=eng)


def build_nc(n_samples, spt=32, x_bufs=3, o_bufs=2, loop_repeats=0,
             pool_l5=(1,), emit_compute=True, emit_out_dma=True):
    """Per-core program: x [nt*P, 126*spt] fp16 -> out [nt*2, P, ...] fp16.

    loop_repeats > 0 wraps the whole body in a hardware For_i loop that
    re-executes it that many times on the same buffers (timing builds only).
    emit_compute/emit_out_dma=False build crippled variants for bottleneck
    isolation (DMA floor / compute ceiling).
    """
    s2 = spt // 2
    tile_samples = P * spt
    assert n_samples % tile_samples == 0, (n_samples, tile_samples)
    nt = n_samples // tile_samples
    xcols = CIN * s2 * H * W * 2       # 126 * spt
    ocols = 12 * s2 * H * W * 2        # per half

    nc = bacc.Bacc(None, target_bir_lowering=False)
    x_d = nc.dram_tensor("x", [nt * P, xcols], F16, kind="ExternalInput")
    o_d = nc.dram_tensor("out", [nt * 2, P, ocols], F16, kind="ExternalOutput")

    with TileContext(nc) as tc:
        with (
            tc.tile_pool(name="xp", bufs=x_bufs) as xp,
            tc.tile_pool(name="op", bufs=o_bufs) as op,
        ):
            def body():
                em = BassEmitter(nc)
                for t in range(nt):
                    xt = xp.tile([P, CIN, s2, H, 2 * W], F16, name="xt")
                    nc.sync.dma_start(out=xt, in_=x_d[t * P : (t + 1) * P, :])
                    ot = op.tile([P, 24, s2, H, 2 * W], F16, name="ot")
                    if emit_compute:
                        emit_tile(em, xt, ot, pool_l5)
                    else:  # keep a data dep so the pipeline shape survives
                        nc.vector.tensor_copy(out=ot[:, 0], in_=xt[:, 0])
                    if emit_out_dma:
                        nc.sync.dma_start(out=o_d[2 * t], in_=ot[:, 0:12])
                        nc.sync.dma_start(out=o_d[2 * t + 1], in_=ot[:, 12:24])

            if loop_repeats > 0:
                with tc.For_i(0, loop_repeats, 1):
                    body()
            else:
                body()

    nc.compile()
    return nc


_NC_CACHE = {}


def _get_nc(n_samples, **kw):
    key = (n_samples, tuple(sorted(kw.items())))
    if key not in _NC_CACHE:
        _NC_CACHE[key] = build_nc(n_samples, **kw)
    return _NC_CACHE[key]


def pack_x(x, n_cores=N_CORES, spt=32):
    """x [N, 3, 6, 7] -> per-core packed fp16 [nt*P, C*s2*H*W*2].

    Per partition: [C, s2, H, W, pair] with the sample pair innermost."""
    n = x.shape[0]
    per = n // n_cores
    s2 = spt // 2
    nt = per // (P * spt)
    xr = np.ascontiguousarray(x, dtype=np.float16).reshape(
        n_cores, nt, P, s2, 2, CIN, H, W)
    xr = xr.transpose(0, 1, 2, 5, 3, 6, 7, 4)  # -> core,t,p,C,s2,H,W,pair
    return np.ascontiguousarray(xr).reshape(n_cores, nt * P, CIN * s2 * HW * 2)


def unpack_out(res_list, spt=32):
    """Per-core out [nt*2, P, 12*s2*H*W*2] fp16 -> conv fp16 [N, 24, 6, 7]."""
    s2 = spt // 2
    o = np.stack([r["out"] for r in res_list])      # [8, nt*2, P, ocols]
    ncores, nt2 = o.shape[0], o.shape[1]
    nt = nt2 // 2
    o = o.reshape(ncores, nt, 2, P, 12, s2, H, W, 2)
    # dims: core,t,half,p,ch,j,h,w,e -> core,t,p,j,e,half,ch,h,w
    o = o.transpose(0, 1, 3, 5, 8, 2, 4, 6, 7)
    return np.ascontiguousarray(o).reshape(ncores * nt * P * s2 * 2, 24, H, W)


def run(x, n_cores=N_CORES, spt=32, **spmd_kwargs):
    """Run on hardware; x full fp32 batch -> full fp32 output."""
    x = np.asarray(x)
    n = x.shape[0]
    xp = pack_x(x, n_cores, spt)
    nc = _get_nc(n // n_cores, spt=spt)
    in_maps = [{"x": xp[c]} for c in range(n_cores)]
    res = run_bass_kernel_spmd(
        nc, in_maps, core_ids=list(range(n_cores)), **spmd_kwargs)
    conv = unpack_out(res.results, spt)
    out = np.empty((n, 3 + 24, H, W), dtype=np.float32)
    out[:, :3] = np.asarray(x, dtype=np.float32).reshape(n, CIN, H, W)
    out[:, 3:] = conv
    return out, res


def kernel(x, **unused_filts):
    """Entry point: full inputs in, full fp32 output out. The filters are the
    fixed 0/1 line patterns from the problem definition (hardcoded)."""
    out, _ = run(x)
    return out


# ---------------------------------------------------------------- emulation

def emulate(x, spt=32):
    """Pure-numpy emulation of the full device+host path (fp16 rounding)."""
    n = x.shape[0]
    s2 = spt // 2
    xp = pack_x(x, N_CORES, spt)
    res = []
    for c in range(N_CORES):
        nt = xp.shape[1] // P
        o_d = np.zeros((nt * 2, P, 12 * s2 * HW * 2), np.float16)
        em = NpEmitter()
        for t in range(nt):
            xt = xp[c, t * P : (t + 1) * P].reshape(P, CIN, s2, H, 2 * W)
            ot = np.zeros((P, 24, s2, H, 2 * W), np.float16)
            emit_tile(em, xt, ot)
            o_d[2 * t] = ot[:, 0:12].reshape(P, -1)
            o_d[2 * t + 1] = ot[:, 12:24].reshape(P, -1)
        res.append({"out": o_d})
    conv = unpack_out(res, spt)
    out = np.empty((n, 27, H, W), np.float32)
    out[:, :3] = np.asarray(x, dtype=np.float32).reshape(n, CIN, H, W)
    out[:, 3:] = conv
    return out


# revision 12
# speedup vs baseline: 18.2948x; 18.2948x over previous
"""Trainium2 Bass kernel for nn_ConnectFourFeatures.

Reference computes out = concat([x] + [conv(x, f) for f in 8 filters], axis=1)
on x [131072, 3, 6, 7] fp32; each filter is channel-diagonal with 0/1 taps on a
line (horiz/vert/diag/anti, run lengths 3 and 5). Output [131072, 27, 6, 7].

Strategy (v2):
 - Pure data-parallel over 8 NeuronCores (batch sharded), no collectives.
 - fp16 end-to-end on device (correctness gate is 2e-2 rel; fp16 gives ~4e-4).
   Host converts x -> fp16 and upcasts the result; halves all HBM traffic.
 - Identity channels (0:3) are never touched on device: the host writes the
   exact fp32 x into the output. Device computes only the 24 conv channels.
 - Sample-PAIR interleaved innermost: SBUF layout [P, C, s2, H, W*2] where the
   last dim holds (w, pair) merged. Every shifted add is then a 3D free AP
   [C*s2, hs, 2*ws] (DVE ISA limit) whose innermost dim is step-1, >=2 elems,
   at even element offsets -> DVE 2x packed mode for 2-byte dtypes.
 - Per direction d: out[R+] = x + x[+d]; complement rows/cols/corners get
   their own small ops; out[R+ & R-] += x[-d]. l=5 filters: ACT base copy of
   the l=3 block + two in-place adds (+-2d). Vertical l5 adds go to GpSimd to
   offload DVE.
"""

import numpy as np

import concourse.bass as bass
import concourse.mybir as mybir
from concourse import bacc
from concourse.bass_utils import run_bass_kernel_spmd
from concourse.tile import TileContext

N_CORES = 8
H, W = 6, 7
CIN = 3
HW = H * W  # 42
P = 128
DIRS = [(0, 1), (1, 0), (1, 1), (1, -1)]  # horiz, vert, diag, anti
F16 = mybir.dt.float16


def _region(dh, dw):
    """(h, w) slices of out cells whose tap x[h+dh, w+dw] is in bounds."""
    return (slice(max(0, -dh), H - max(0, dh)),
            slice(max(0, -dw), W - max(0, dw)))


def _sh(s, d):
    return slice(s.start + d, s.stop + d)


def _w2(ws, dw=0):
    """w-range slice in the merged (w, pair) coordinate, shifted by dw."""
    return slice(2 * (ws.start + dw), 2 * (ws.stop + dw))


def _isect(a, b):
    return slice(max(a.start, b.start), min(a.stop, b.stop))


class NpEmitter:
    """Numpy emulation of the per-tile program (fp16 semantics like DVE)."""

    def add(self, out, a, b, eng="v"):
        out[...] = a + b

    def copy(self, out, a, eng="v"):
        out[...] = a


class BassEmitter:
    def __init__(self, nc):
        self.nc = nc

    def add(self, out, a, b, eng="v"):
        e = self.nc.gpsimd if eng == "g" else self.nc.vector
        e.tensor_add(out=out, in0=a, in1=b)

    def copy(self, out, a, eng="v"):
        if eng == "s":
            self.nc.scalar.copy(out=out, in_=a)
        else:
            self.nc.vector.tensor_copy(out=out, in_=a)


def emit_tile(em, xt, ot, pool_l5=(), copy_plan="ssvv", corner_eng="v",
              group_copies=True, after_l3=None):
    """xt [P, 3, s2, H, 14] -> ot [P, 24, s2, H, 14] (l3 chans 0:12, l5 chans
    12:24, direction order horiz/vert/diag/anti; (w, pair) merged last).

    copy_plan: engine per direction for the l5 base copy ('s'=ACT, 'v'=DVE
    4x-packed). group_copies: emit all base copies after the l3 loop (True)
    vs interleaved per direction (False)."""
    def base_copy(i):
        em.copy(ot[:, 12 + 3 * i : 15 + 3 * i], ot[:, 3 * i : 3 * i + 3],
                eng=copy_plan[i])

    # l=3 filters
    for i, (dh, dw) in enumerate(DIRS):
        b3 = ot[:, 3 * i : 3 * i + 3]
        hp, wp = _region(dh, dw)
        hm, wm = _region(-dh, -dw)
        # op1: fresh write covering R+ : x + x[+d]
        em.add(b3[:, :, :, hp, _w2(wp)], xt[:, :, :, hp, _w2(wp)],
               xt[:, :, :, _sh(hp, dh), _w2(wp, dw)])
        # op2: complement of R+ gets x + x[-d] (corners: copy of x)
        if dh == 0:
            wc = slice(W - 1, W) if dw > 0 else slice(0, 1)
            em.add(b3[:, :, :, :, _w2(wc)], xt[:, :, :, :, _w2(wc)],
                   xt[:, :, :, :, _w2(wc, -dw)])
        elif dw == 0:
            hc = slice(H - 1, H) if dh > 0 else slice(0, 1)
            em.add(b3[:, :, :, hc, :], xt[:, :, :, hc, :],
                   xt[:, :, :, _sh(hc, -dh), :])
        else:
            hc = slice(H - 1, H)  # dh == 1 for both diagonal patterns
            wc = slice(W - 1, W) if dw > 0 else slice(0, 1)
            wr = slice(max(0, dw), W + min(0, dw))
            em.add(b3[:, :, :, hc, _w2(wr)], xt[:, :, :, hc, _w2(wr)],
                   xt[:, :, :, _sh(hc, -dh), _w2(wr, -dw)])
            hr = slice(1, H - 1)
            em.add(b3[:, :, :, hr, _w2(wc)], xt[:, :, :, hr, _w2(wc)],
                   xt[:, :, :, _sh(hr, -dh), _w2(wc, -dw)])
            oc = slice(W - 1 - wc.start, W - wc.start)  # untapped corners
            em.copy(b3[:, :, :, hc, _w2(oc)], xt[:, :, :, hc, _w2(oc)],
                    eng=corner_eng)
            em.copy(b3[:, :, :, 0:1, _w2(wc)], xt[:, :, :, 0:1, _w2(wc)],
                    eng=corner_eng)
        # op3: += x[-d] where both taps exist
        hi, wi = _isect(hp, hm), _isect(wp, wm)
        em.add(b3[:, :, :, hi, _w2(wi)], b3[:, :, :, hi, _w2(wi)],
               xt[:, :, :, _sh(hi, -dh), _w2(wi, -dw)])
        if not group_copies:
            base_copy(i)

    if after_l3 is not None:
        after_l3()
    if group_copies:
        for i in range(4):
            base_copy(i)

    # l=5: += x[+-2d] on top of the copied l3 block
    for i, (dh0, dw0) in enumerate(DIRS):
        b5 = ot[:, 12 + 3 * i : 15 + 3 * i]
        eng = "g" if i in pool_l5 else "v"
        for sgn in (2, -2):
            dh, dw = sgn * dh0, sgn * dw0
            hs, ws = _region(dh, dw)
            em.add(b5[:, :, :, hs, _w2(ws)], b5[:, :, :, hs, _w2(ws)],
                   xt[:, :, :, _sh(hs, dh), _w2(ws, dw)], eng=eng)


def build_nc(n_samples, spt=32, x_bufs=3, o_bufs=2, loop_repeats=0,
             pool_l5=(), copy_plan="ssvv", corner_eng="v", group_copies=True,
             early_half0=False, emit_compute=True, emit_out_dma=True):
    """Per-core program: x [nt*P, 126*spt] fp16 -> out [nt*2, P, ...] fp16.

    loop_repeats > 0 wraps the whole body in a hardware For_i loop that
    re-executes it that many times on the same buffers (timing builds only).
    emit_compute/emit_out_dma=False build crippled variants for bottleneck
    isolation (DMA floor / compute ceiling).
    """
    s2 = spt // 2
    tile_samples = P * spt
    assert n_samples % tile_samples == 0, (n_samples, tile_samples)
    nt = n_samples // tile_samples
    xcols = CIN * s2 * H * W * 2       # 126 * spt
    ocols = 12 * s2 * H * W * 2        # per half

    nc = bacc.Bacc(None, target_bir_lowering=False)
    x_d = nc.dram_tensor("x", [nt * P, xcols], F16, kind="ExternalInput")
    o_d = nc.dram_tensor("out", [nt * 2, P, ocols], F16, kind="ExternalOutput")

    with TileContext(nc) as tc:
        with (
            tc.tile_pool(name="xp", bufs=x_bufs) as xp,
            tc.tile_pool(name="op", bufs=o_bufs) as op,
        ):
            def body():
                em = BassEmitter(nc)
                for t in range(nt):
                    xt = xp.tile([P, CIN, s2, H, 2 * W], F16, name="xt")
                    nc.sync.dma_start(out=xt, in_=x_d[t * P : (t + 1) * P, :])
                    ot = op.tile([P, 24, s2, H, 2 * W], F16, name="ot")
                    half0 = (lambda t=t, ot=ot: nc.sync.dma_start(
                        out=o_d[2 * t], in_=ot[:, 0:12]))
                    if emit_compute:
                        emit_tile(em, xt, ot, pool_l5, copy_plan, corner_eng,
                                  group_copies,
                                  after_l3=half0 if (early_half0 and
                                                    emit_out_dma) else None)
                    else:  # keep a data dep so the pipeline shape survives
                        nc.vector.tensor_copy(out=ot[:, 0], in_=xt[:, 0])
                    if emit_out_dma:
                        if not (early_half0 and emit_compute):
                            half0()
                        nc.sync.dma_start(out=o_d[2 * t + 1], in_=ot[:, 12:24])

            if loop_repeats > 0:
                with tc.For_i(0, loop_repeats, 1):
                    body()
            else:
                body()

    nc.compile()
    return nc


_NC_CACHE = {}


def _get_nc(n_samples, **kw):
    key = (n_samples, tuple(sorted(kw.items())))
    if key not in _NC_CACHE:
        _NC_CACHE[key] = build_nc(n_samples, **kw)
    return _NC_CACHE[key]


def pack_x(x, n_cores=N_CORES, spt=32):
    """x [N, 3, 6, 7] -> per-core packed fp16 [nt*P, C*s2*H*W*2].

    Per partition: [C, s2, H, W, pair] with the sample pair innermost."""
    n = x.shape[0]
    per = n // n_cores
    s2 = spt // 2
    nt = per // (P * spt)
    xr = np.ascontiguousarray(x, dtype=np.float16).reshape(
        n_cores, nt, P, s2, 2, CIN, H, W)
    xr = xr.transpose(0, 1, 2, 5, 3, 6, 7, 4)  # -> core,t,p,C,s2,H,W,pair
    return np.ascontiguousarray(xr).reshape(n_cores, nt * P, CIN * s2 * HW * 2)


def unpack_out(res_list, spt=32):
    """Per-core out [nt*2, P, 12*s2*H*W*2] fp16 -> conv fp16 [N, 24, 6, 7]."""
    s2 = spt // 2
    o = np.stack([r["out"] for r in res_list])      # [8, nt*2, P, ocols]
    ncores, nt2 = o.shape[0], o.shape[1]
    nt = nt2 // 2
    o = o.reshape(ncores, nt, 2, P, 12, s2, H, W, 2)
    # dims: core,t,half,p,ch,j,h,w,e -> core,t,p,j,e,half,ch,h,w
    o = o.transpose(0, 1, 3, 5, 8, 2, 4, 6, 7)
    return np.ascontiguousarray(o).reshape(ncores * nt * P * s2 * 2, 24, H, W)


def run(x, n_cores=N_CORES, spt=32, **spmd_kwargs):
    """Run on hardware; x full fp32 batch -> full fp32 output."""
    x = np.asarray(x)
    n = x.shape[0]
    xp = pack_x(x, n_cores, spt)
    nc = _get_nc(n // n_cores, spt=spt)
    in_maps = [{"x": xp[c]} for c in range(n_cores)]
    res = run_bass_kernel_spmd(
        nc, in_maps, core_ids=list(range(n_cores)), **spmd_kwargs)
    conv = unpack_out(res.results, spt)
    out = np.empty((n, 3 + 24, H, W), dtype=np.float32)
    out[:, :3] = np.asarray(x, dtype=np.float32).reshape(n, CIN, H, W)
    out[:, 3:] = conv
    return out, res


def kernel(x, **unused_filts):
    """Entry point: full inputs in, full fp32 output out. The filters are the
    fixed 0/1 line patterns from the problem definition (hardcoded)."""
    out, _ = run(x)
    return out


# ---------------------------------------------------------------- emulation

def emulate(x, spt=32):
    """Pure-numpy emulation of the full device+host path (fp16 rounding)."""
    n = x.shape[0]
    s2 = spt // 2
    xp = pack_x(x, N_CORES, spt)
    res = []
    for c in range(N_CORES):
        nt = xp.shape[1] // P
        o_d = np.zeros((nt * 2, P, 12 * s2 * HW * 2), np.float16)
        em = NpEmitter()
        for t in range(nt):
            xt = xp[c, t * P : (t + 1) * P].reshape(P, CIN, s2, H, 2 * W)
            ot = np.zeros((P, 24, s2, H, 2 * W), np.float16)
            emit_tile(em, xt, ot)
            o_d[2 * t] = ot[:, 0:12].reshape(P, -1)
            o_d[2 * t + 1] = ot[:, 12:24].reshape(P, -1)
        res.append({"out": o_d})
    conv = unpack_out(res, spt)
    out = np.empty((n, 27, H, W), np.float32)
    out[:, :3] = np.asarray(x, dtype=np.float32).reshape(n, CIN, H, W)
    out[:, 3:] = conv
    return out


# revision 13
# speedup vs baseline: 19.9598x; 1.0910x over previous
"""Trainium2 Bass kernel for nn_ConnectFourFeatures.

Reference computes out = concat([x] + [conv(x, f) for f in 8 filters], axis=1)
on x [131072, 3, 6, 7] fp32; each filter is channel-diagonal with 0/1 taps on a
line (horiz/vert/diag/anti, run lengths 3 and 5). Output [131072, 27, 6, 7].

Strategy (v2):
 - Pure data-parallel over 8 NeuronCores (batch sharded), no collectives.
 - fp16 end-to-end on device (correctness gate is 2e-2 rel; fp16 gives ~4e-4).
   Host converts x -> fp16 and upcasts the result; halves all HBM traffic.
 - Identity channels (0:3) are never touched on device: the host writes the
   exact fp32 x into the output. Device computes only the 24 conv channels.
 - Sample-PAIR interleaved innermost: SBUF layout [P, C, s2, H, W*2] where the
   last dim holds (w, pair) merged. Every shifted add is then a 3D free AP
   [C*s2, hs, 2*ws] (DVE ISA limit) whose innermost dim is step-1, >=2 elems,
   at even element offsets -> DVE 2x packed mode for 2-byte dtypes.
 - Per direction d: out[R+] = x + x[+d]; complement rows/cols/corners get
   their own small ops; out[R+ & R-] += x[-d]. l=5 filters: base copy of the
   l=3 block (split ACT/DVE to avoid cross-engine stalls) + two in-place DVE
   adds (+-2d). GpSimd measured slower than DVE for 2-input adds — unused.

Measured (single-core For_i repeat-delta, R=8 vs 2080): ~127-139 us per core
per pass vs a ~85 us DMA floor (37.2 MB at fabric rate) and ~109 us DVE floor.
"""

import numpy as np

import concourse.bass as bass
import concourse.mybir as mybir
from concourse import bacc
from concourse.bass_utils import run_bass_kernel_spmd
from concourse.tile import TileContext

N_CORES = 8
H, W = 6, 7
CIN = 3
HW = H * W  # 42
P = 128
DIRS = [(0, 1), (1, 0), (1, 1), (1, -1)]  # horiz, vert, diag, anti
F16 = mybir.dt.float16


def _region(dh, dw):
    """(h, w) slices of out cells whose tap x[h+dh, w+dw] is in bounds."""
    return (slice(max(0, -dh), H - max(0, dh)),
            slice(max(0, -dw), W - max(0, dw)))


def _sh(s, d):
    return slice(s.start + d, s.stop + d)


def _w2(ws, dw=0):
    """w-range slice in the merged (w, pair) coordinate, shifted by dw."""
    return slice(2 * (ws.start + dw), 2 * (ws.stop + dw))


def _isect(a, b):
    return slice(max(a.start, b.start), min(a.stop, b.stop))


class NpEmitter:
    """Numpy emulation of the per-tile program (fp16 semantics like DVE)."""

    def add(self, out, a, b, eng="v"):
        out[...] = a + b

    def copy(self, out, a, eng="v"):
        out[...] = a


class BassEmitter:
    def __init__(self, nc):
        self.nc = nc

    def add(self, out, a, b, eng="v"):
        e = self.nc.gpsimd if eng == "g" else self.nc.vector
        e.tensor_add(out=out, in0=a, in1=b)

    def copy(self, out, a, eng="v"):
        if eng == "s":
            self.nc.scalar.copy(out=out, in_=a)
        else:
            self.nc.vector.tensor_copy(out=out, in_=a)


def emit_tile(em, xt, ot, pool_l5=(), copy_plan="ssvv", corner_eng="v",
              group_copies=True, after_l3=None):
    """xt [P, 3, s2, H, 14] -> ot [P, 24, s2, H, 14] (l3 chans 0:12, l5 chans
    12:24, direction order horiz/vert/diag/anti; (w, pair) merged last).

    copy_plan: engine per direction for the l5 base copy ('s'=ACT, 'v'=DVE
    4x-packed). group_copies: emit all base copies after the l3 loop (True)
    vs interleaved per direction (False)."""
    def base_copy(i):
        em.copy(ot[:, 12 + 3 * i : 15 + 3 * i], ot[:, 3 * i : 3 * i + 3],
                eng=copy_plan[i])

    # l=3 filters
    for i, (dh, dw) in enumerate(DIRS):
        b3 = ot[:, 3 * i : 3 * i + 3]
        hp, wp = _region(dh, dw)
        hm, wm = _region(-dh, -dw)
        # op1: fresh write covering R+ : x + x[+d]
        em.add(b3[:, :, :, hp, _w2(wp)], xt[:, :, :, hp, _w2(wp)],
               xt[:, :, :, _sh(hp, dh), _w2(wp, dw)])
        # op2: complement of R+ gets x + x[-d] (corners: copy of x)
        if dh == 0:
            wc = slice(W - 1, W) if dw > 0 else slice(0, 1)
            em.add(b3[:, :, :, :, _w2(wc)], xt[:, :, :, :, _w2(wc)],
                   xt[:, :, :, :, _w2(wc, -dw)])
        elif dw == 0:
            hc = slice(H - 1, H) if dh > 0 else slice(0, 1)
            em.add(b3[:, :, :, hc, :], xt[:, :, :, hc, :],
                   xt[:, :, :, _sh(hc, -dh), :])
        else:
            hc = slice(H - 1, H)  # dh == 1 for both diagonal patterns
            wc = slice(W - 1, W) if dw > 0 else slice(0, 1)
            wr = slice(max(0, dw), W + min(0, dw))
            em.add(b3[:, :, :, hc, _w2(wr)], xt[:, :, :, hc, _w2(wr)],
                   xt[:, :, :, _sh(hc, -dh), _w2(wr, -dw)])
            hr = slice(1, H - 1)
            em.add(b3[:, :, :, hr, _w2(wc)], xt[:, :, :, hr, _w2(wc)],
                   xt[:, :, :, _sh(hr, -dh), _w2(wc, -dw)])
            oc = slice(W - 1 - wc.start, W - wc.start)  # untapped corners
            em.copy(b3[:, :, :, hc, _w2(oc)], xt[:, :, :, hc, _w2(oc)],
                    eng=corner_eng)
            em.copy(b3[:, :, :, 0:1, _w2(wc)], xt[:, :, :, 0:1, _w2(wc)],
                    eng=corner_eng)
        # op3: += x[-d] where both taps exist
        hi, wi = _isect(hp, hm), _isect(wp, wm)
        em.add(b3[:, :, :, hi, _w2(wi)], b3[:, :, :, hi, _w2(wi)],
               xt[:, :, :, _sh(hi, -dh), _w2(wi, -dw)])
        if not group_copies:
            base_copy(i)

    if after_l3 is not None:
        after_l3()
    if group_copies:
        for i in range(4):
            base_copy(i)

    # l=5: += x[+-2d] on top of the copied l3 block
    for i, (dh0, dw0) in enumerate(DIRS):
        b5 = ot[:, 12 + 3 * i : 15 + 3 * i]
        eng = "g" if i in pool_l5 else "v"
        for sgn in (2, -2):
            dh, dw = sgn * dh0, sgn * dw0
            hs, ws = _region(dh, dw)
            em.add(b5[:, :, :, hs, _w2(ws)], b5[:, :, :, hs, _w2(ws)],
                   xt[:, :, :, _sh(hs, dh), _w2(ws, dw)], eng=eng)


def build_nc(n_samples, spt=32, x_bufs=3, o_bufs=2, loop_repeats=0,
             pool_l5=(), copy_plan="ssvv", corner_eng="v", group_copies=True,
             early_half0=False, emit_compute=True, emit_out_dma=True):
    """Per-core program: x [nt*P, 126*spt] fp16 -> out [nt*2, P, ...] fp16.

    loop_repeats > 0 wraps the whole body in a hardware For_i loop that
    re-executes it that many times on the same buffers (timing builds only).
    emit_compute/emit_out_dma=False build crippled variants for bottleneck
    isolation (DMA floor / compute ceiling).
    """
    s2 = spt // 2
    tile_samples = P * spt
    assert n_samples % tile_samples == 0, (n_samples, tile_samples)
    nt = n_samples // tile_samples
    xcols = CIN * s2 * H * W * 2       # 126 * spt
    ocols = 12 * s2 * H * W * 2        # per half

    nc = bacc.Bacc(None, target_bir_lowering=False)
    x_d = nc.dram_tensor("x", [nt * P, xcols], F16, kind="ExternalInput")
    o_d = nc.dram_tensor("out", [nt * 2, P, ocols], F16, kind="ExternalOutput")

    with TileContext(nc) as tc:
        with (
            tc.tile_pool(name="xp", bufs=x_bufs) as xp,
            tc.tile_pool(name="op", bufs=o_bufs) as op,
        ):
            def body():
                em = BassEmitter(nc)
                for t in range(nt):
                    xt = xp.tile([P, CIN, s2, H, 2 * W], F16, name="xt")
                    nc.sync.dma_start(out=xt, in_=x_d[t * P : (t + 1) * P, :])
                    ot = op.tile([P, 24, s2, H, 2 * W], F16, name="ot")
                    half0 = (lambda t=t, ot=ot: nc.sync.dma_start(
                        out=o_d[2 * t], in_=ot[:, 0:12]))
                    if emit_compute:
                        emit_tile(em, xt, ot, pool_l5, copy_plan, corner_eng,
                                  group_copies,
                                  after_l3=half0 if (early_half0 and
                                                    emit_out_dma) else None)
                    else:  # keep a data dep so the pipeline shape survives
                        nc.vector.tensor_copy(out=ot[:, 0], in_=xt[:, 0])
                    if emit_out_dma:
                        if not (early_half0 and emit_compute):
                            half0()
                        nc.sync.dma_start(out=o_d[2 * t + 1], in_=ot[:, 12:24])

            if loop_repeats > 0:
                with tc.For_i(0, loop_repeats, 1):
                    body()
            else:
                body()

    nc.compile()
    return nc


_NC_CACHE = {}


def _get_nc(n_samples, **kw):
    key = (n_samples, tuple(sorted(kw.items())))
    if key not in _NC_CACHE:
        _NC_CACHE[key] = build_nc(n_samples, **kw)
    return _NC_CACHE[key]


def pack_x(x, n_cores=N_CORES, spt=32):
    """x [N, 3, 6, 7] -> per-core packed fp16 [nt*P, C*s2*H*W*2].

    Per partition: [C, s2, H, W, pair] with the sample pair innermost."""
    n = x.shape[0]
    per = n // n_cores
    s2 = spt // 2
    nt = per // (P * spt)
    xr = np.ascontiguousarray(x, dtype=np.float16).reshape(
        n_cores, nt, P, s2, 2, CIN, H, W)
    xr = xr.transpose(0, 1, 2, 5, 3, 6, 7, 4)  # -> core,t,p,C,s2,H,W,pair
    return np.ascontiguousarray(xr).reshape(n_cores, nt * P, CIN * s2 * HW * 2)


def unpack_out(res_list, spt=32):
    """Per-core out [nt*2, P, 12*s2*H*W*2] fp16 -> conv fp16 [N, 24, 6, 7]."""
    s2 = spt // 2
    o = np.stack([r["out"] for r in res_list])      # [8, nt*2, P, ocols]
    ncores, nt2 = o.shape[0], o.shape[1]
    nt = nt2 // 2
    o = o.reshape(ncores, nt, 2, P, 12, s2, H, W, 2)
    # dims: core,t,half,p,ch,j,h,w,e -> core,t,p,j,e,half,ch,h,w
    o = o.transpose(0, 1, 3, 5, 8, 2, 4, 6, 7)
    return np.ascontiguousarray(o).reshape(ncores * nt * P * s2 * 2, 24, H, W)


def run(x, n_cores=N_CORES, spt=32, **spmd_kwargs):
    """Run on hardware; x full fp32 batch -> full fp32 output."""
    x = np.asarray(x)
    n = x.shape[0]
    xp = pack_x(x, n_cores, spt)
    nc = _get_nc(n // n_cores, spt=spt)
    in_maps = [{"x": xp[c]} for c in range(n_cores)]
    res = run_bass_kernel_spmd(
        nc, in_maps, core_ids=list(range(n_cores)), **spmd_kwargs)
    conv = unpack_out(res.results, spt)
    out = np.empty((n, 3 + 24, H, W), dtype=np.float32)
    out[:, :3] = np.asarray(x, dtype=np.float32).reshape(n, CIN, H, W)
    out[:, 3:] = conv
    return out, res


def kernel(x, **unused_filts):
    """Entry point: full inputs in, full fp32 output out. The filters are the
    fixed 0/1 line patterns from the problem definition (hardcoded)."""
    out, _ = run(x)
    return out


# ---------------------------------------------------------------- emulation

def emulate(x, spt=32):
    """Pure-numpy emulation of the full device+host path (fp16 rounding)."""
    n = x.shape[0]
    s2 = spt // 2
    xp = pack_x(x, N_CORES, spt)
    res = []
    for c in range(N_CORES):
        nt = xp.shape[1] // P
        o_d = np.zeros((nt * 2, P, 12 * s2 * HW * 2), np.float16)
        em = NpEmitter()
        for t in range(nt):
            xt = xp[c, t * P : (t + 1) * P].reshape(P, CIN, s2, H, 2 * W)
            ot = np.zeros((P, 24, s2, H, 2 * W), np.float16)
            emit_tile(em, xt, ot)
            o_d[2 * t] = ot[:, 0:12].reshape(P, -1)
            o_d[2 * t + 1] = ot[:, 12:24].reshape(P, -1)
        res.append({"out": o_d})
    conv = unpack_out(res, spt)
    out = np.empty((n, 27, H, W), np.float32)
    out[:, :3] = np.asarray(x, dtype=np.float32).reshape(n, CIN, H, W)
    out[:, 3:] = conv
    return out
